# revision 3
# baseline (speedup 1.0000x reference)
"""Trainium2 Bass kernel for the DiffPool-style GCN forward pass.

Computation (dead softmax/pool branches of the reference are skipped — their
outputs are unused):
    x1 = relu(Dhalf (A+I) Dhalf (x @ W1e) + b1e)
    x2 = relu(Dhalf (A+I) Dhalf (x1 @ W2e) + b2e)
    out = (graph_mean_pool(x2) @ Wlin) + blin          -> [64, 10] fp32

Normalization folds into node-level row scalings: with h' = dinv * (x @ W),
agg = dinv * (scatter_sum(h'[src] -> dst) + h') + b.

Distribution: nodes (and incident edges, bucketed by dst) are sharded over the
8 NeuronCores; the 128x128 weights are replicated; full h' tables are built
with an AllGather after each dense phase; per-graph mean-pool partial sums are
combined with an AllReduce.

Per-core edge pipeline: edges are sorted by dst into 128-node aligned windows;
rows h'[src] are fetched with batched dma_gather (1024 rows/call, 4 SWDGE
queues); a one-hot [edges x window-node] matrix built on DVE via broadcast
is_equal turns the scatter-add into PE matmuls accumulating in PSUM.
"""

import numpy as np

N = 50000
E = 800000
G = 64
C = 128
C_OUT = 10
NCORES = 8
NLOC = N // NCORES          # 6250
W = (NLOC + 127) // 128     # 49 windows of 128 dst nodes
NPAD = W * 128              # 6272
HALF = 3125                 # rows per core per half-table (int16 index limit);
                            # lo table = AllGather of each core's first 3125 rows
MAX_CALL_CHUNKS = 8         # 1024 rows per dma_gather call
NQ = 4                      # SWDGE queues

_CACHE = {}


def _build_program(a_chunks, b_chunks, call_plan, total_chunks, total_idxcols):
    """Build + compile the SPMD Bass program. Structure args are
    per-window chunk counts (uniform across cores) and the gather call plan."""
    import concourse.bass as bass
    import concourse.bacc as bacc
    import concourse.mybir as mybir
    import concourse.tile as tile
    from concourse import library_config
    from concourse.bass_interp import get_hw_module
    from concourse.tile_rust import add_dep_helper
    from concourse.masks import make_identity

    f32 = mybir.dt.float32
    i16 = mybir.dt.int16

    nc = bacc.Bacc("TRN2", target_bir_lowering=False, debug=False,
                   num_devices=NCORES, num_swdge_queues=NQ)

    # ---- I/O ----
    xT_in = nc.dram_tensor("xT", [C, NPAD], f32, kind="ExternalInput")
    idx_in = nc.dram_tensor("idx16", [C, total_idxcols], i16, kind="ExternalInput")
    drel_in = nc.dram_tensor("drel", [C, total_chunks], f32, kind="ExternalInput")
    iota_in = nc.dram_tensor("iota", [C, C], f32, kind="ExternalInput")
    dinv_in = nc.dram_tensor("dinvw", [C, W], f32, kind="ExternalInput")
    bcol_in = nc.dram_tensor("batchcol", [C, W], f32, kind="ExternalInput")
    b1_in = nc.dram_tensor("bias1t", [C, C], f32, kind="ExternalInput")
    b2_in = nc.dram_tensor("bias2t", [C, C], f32, kind="ExternalInput")
    w1_in = nc.dram_tensor("w1e", [C, C], f32, kind="ExternalInput")
    w2_in = nc.dram_tensor("w2e", [C, C], f32, kind="ExternalInput")
    wlin_in = nc.dram_tensor("wlin", [C, C_OUT], f32, kind="ExternalInput")
    blin_in = nc.dram_tensor("blinb", [G, C_OUT], f32, kind="ExternalInput")
    icnt_in = nc.dram_tensor("invcnt", [G, 1], f32, kind="ExternalInput")
    out_t = nc.dram_tensor("out", [G, C_OUT], f32, kind="ExternalOutput")

    NTAIL = NLOC - (W - 1) * 128  # valid rows in last window (106)

    with tile.TileContext(nc) as tc:
        with tc.tile_pool(name="res", bufs=1) as res, \
             tc.tile_pool(name="gp", bufs=6) as gp, \
             tc.tile_pool(name="ohp", bufs=6) as ohp, \
             tc.tile_pool(name="tmp", bufs=6) as tmpp, \
             tc.tile_pool(name="x2p", bufs=4) as x2p, \
             tc.tile_pool(name="selp", bufs=4) as selp, \
             tc.tile_pool(name="psw", bufs=3, space="PSUM") as psw, \
             tc.tile_pool(name="psd", bufs=2, space="PSUM") as psd, \
             tc.tile_pool(name="pst", bufs=1, space="PSUM") as pst, \
             tc.tile_pool(name="dram", bufs=1, space="DRAM") as dram:

            lib = nc.gpsimd.load_library(library_config.mlp)

            # ---- residents ----
            xT = res.tile([C, NPAD], f32)
            nc.sync.dma_start(out=xT[:], in_=xT_in[:])
            idx16 = res.tile([C, total_idxcols], i16)
            nc.sync.dma_start(out=idx16[:], in_=idx_in[:])
            drel = res.tile([C, total_chunks], f32)
            nc.sync.dma_start(out=drel[:], in_=drel_in[:])
            iota = res.tile([C, C], f32)
            nc.sync.dma_start(out=iota[:], in_=iota_in[:])
            dinvw = res.tile([C, W], f32)
            nc.sync.dma_start(out=dinvw[:], in_=dinv_in[:])
            bcol = res.tile([C, W], f32)
            nc.sync.dma_start(out=bcol[:], in_=bcol_in[:])
            bias1 = res.tile([C, C], f32)
            nc.sync.dma_start(out=bias1[:], in_=b1_in[:])
            bias2 = res.tile([C, C], f32)
            nc.sync.dma_start(out=bias2[:], in_=b2_in[:])
            w1 = res.tile([C, C], f32)
            nc.sync.dma_start(out=w1[:], in_=w1_in[:])
            w2 = res.tile([C, C], f32)
            nc.sync.dma_start(out=w2[:], in_=w2_in[:])
            wlin = res.tile([C, C_OUT], f32)
            nc.sync.dma_start(out=wlin[:], in_=wlin_in[:])
            blinb = res.tile([G, C_OUT], f32)
            nc.sync.dma_start(out=blinb[:], in_=blin_in[:])
            icnt = res.tile([G, 1], f32)
            nc.sync.dma_start(out=icnt[:], in_=icnt_in[:])
            ident = res.tile([C, C], f32)
            make_identity(nc, ident[:])

            h1res = res.tile([C, NPAD], f32)   # h1' = dinv*(x@W1e), node-major blocks
            x1res = res.tile([C, NPAD], f32)
            h2res = res.tile([C, NPAD], f32)

            # ---- collective buffers ----
            # dma_gather mishandles nonzero source offsets, so each layer's
            # table is built as TWO offset-0 AllGather outputs (lo/hi halves
            # of every core's shard).
            ag1l_in = dram.tile([HALF, C], f32)
            ag1l_out = dram.tile([HALF * NCORES, C], f32)
            ag1h_in = dram.tile([HALF, C], f32)
            ag1h_out = dram.tile([HALF * NCORES, C], f32)
            ag2l_in = dram.tile([HALF, C], f32)
            ag2l_out = dram.tile([HALF * NCORES, C], f32)
            ag2h_in = dram.tile([HALF, C], f32)
            ag2h_out = dram.tile([HALF * NCORES, C], f32)
            ar_in = dram.tile([C, G], f32)
            ar_out = dram.tile([C, G], f32)

            def dense_phase(src_res, wt, layer, hres, ag_lo, ag_hi):
                """hres[:, b*128:...] = dinv * (src @ W); also DMA to ag_in.
                Layer 1 reads xT directly (src_res is chan-major); layer 2
                transposes x1res blocks via PE."""
                for b in range(W):
                    cols = slice(b * 128, (b + 1) * 128)
                    if layer == 1:
                        lhsT = src_res[:, cols]
                    else:
                        pst_t = psd.tile([C, C], f32, space="PSUM", tag="tps")
                        nc.tensor.transpose(out=pst_t[:], in_=src_res[:, cols],
                                            identity=ident[:])
                        xt_s = tmpp.tile([C, C], f32, tag="xts")
                        nc.vector.tensor_copy(out=xt_s[:], in_=pst_t[:])
                        lhsT = xt_s[:]
                    ps = psd.tile([C, C], f32, space="PSUM", tag="pd")
                    nc.tensor.matmul(out=ps[:], lhsT=lhsT, rhs=wt[:],
                                     start=True, stop=True)
                    nc.vector.tensor_scalar_mul(hres[:, cols], in0=ps[:],
                                                scalar1=dinvw[:, b:b + 1])
                # ship the shard halves to the collective input buffers.
                # local rows [0,3125) -> ag_lo, [3125,6250) -> ag_hi
                blk_a = HALF // 128            # 24 full blocks in lo
                rem_a = HALF - blk_a * 128     # 53
                nc.sync.dma_start(
                    out=ag_lo[0:blk_a * 128, :].rearrange("(b p) c -> p b c", p=128),
                    in_=hres[:, 0:blk_a * 128].rearrange("p (b c) -> p b c", c=C))
                nc.sync.dma_start(
                    out=ag_lo[blk_a * 128:HALF, :],
                    in_=hres[0:rem_a, blk_a * 128:(blk_a + 1) * 128])
                nc.sync.dma_start(
                    out=ag_hi[0:128 - rem_a, :],
                    in_=hres[rem_a:128, blk_a * 128:(blk_a + 1) * 128])
                mid = 128 - rem_a              # 75 rows taken from block 24
                blk_b0 = blk_a + 1             # 25
                nblk_b = W - 1 - blk_b0        # blocks 25..47 (23 full)
                nc.sync.dma_start(
                    out=ag_hi[mid:mid + nblk_b * 128, :].rearrange("(b p) c -> p b c", p=128),
                    in_=hres[:, blk_b0 * 128:(W - 1) * 128].rearrange("p (b c) -> p b c", c=C))
                nc.sync.dma_start(
                    out=ag_hi[mid + nblk_b * 128:HALF, :],
                    in_=hres[0:NTAIL, (W - 1) * 128:W * 128])

            def edge_layer(lo_t, hi_t, hres, layer, bias, outres, ps_pool_acc):
                """One message-passing layer: gather h'[src], one-hot matmul
                scatter into dst windows, epilogue. Layer 1 writes x1res;
                layer 2 makes x2 tiles and accumulates pooling partials."""
                lo = lo_t[:]
                hi = hi_t[:]
                chunk_base = 0
                for w in range(W):
                    calls = call_plan[w]
                    cw = a_chunks[w] + b_chunks[w]
                    # gather calls
                    gtiles = []
                    for (half_sel, ccount, idx_col, qn) in calls:
                        gt = gp.tile([C, MAX_CALL_CHUNKS * C], f32, tag="g")
                        src_ap = lo if half_sel == 0 else hi
                        ni = ccount * 128
                        gi = nc.gpsimd.dma_gather(
                            gt[:, 0:ccount * C].rearrange("p (k d) -> p k d", d=C),
                            src_ap, idx16[:, idx_col:idx_col + ccount * 8],
                            ni, ni, C, single_packet=True, queue_num=qn)
                        add_dep_helper(gi.ins, lib.ins, False, "needs mlp lib")
                        gtiles.append((gt, ccount))
                    # one-hot per call + matmuls accumulating into the window
                    ps = psw.tile([C, C], f32, space="PSUM", tag="pw")
                    j = 0
                    for ci, (gt, ccount) in enumerate(gtiles):
                        oh = ohp.tile([C, MAX_CALL_CHUNKS * C], f32, tag="oh")
                        dcols = drel[:, chunk_base + j:chunk_base + j + ccount]
                        nc.vector.tensor_tensor(
                            out=oh[:, 0:ccount * C].rearrange("p (k m) -> p k m", m=C),
                            in0=dcols.unsqueeze(2).to_broadcast([C, ccount, C]),
                            in1=iota[:].unsqueeze(1).to_broadcast([C, ccount, C]),
                            op=mybir.AluOpType.is_equal)
                        for k in range(ccount):
                            nc.tensor.matmul(
                                out=ps[:],
                                lhsT=oh[:, k * C:(k + 1) * C],
                                rhs=gt[:, k * C:(k + 1) * C],
                                start=(j + k == 0), stop=(j + k == cw - 1))
                        j += ccount
                    chunk_base += cw
                    # epilogue: act = relu(dinv*(S + h') + b)
                    cols = slice(w * 128, (w + 1) * 128)
                    t1 = tmpp.tile([C, C], f32, tag="t1")
                    nc.vector.tensor_add(out=t1[:], in0=ps[:], in1=hres[:, cols])
                    nc.vector.tensor_scalar_mul(t1[:], in0=t1[:],
                                                scalar1=dinvw[:, w:w + 1])
                    nc.vector.tensor_add(out=t1[:], in0=t1[:], in1=bias[:])
                    if layer == 1:
                        nc.scalar.activation(outres[:, cols], t1[:],
                                             mybir.ActivationFunctionType.Relu)
                    else:
                        x2t = x2p.tile([C, C], f32, tag="x2")
                        nc.scalar.activation(x2t[:], t1[:],
                                             mybir.ActivationFunctionType.Relu)
                        sel = selp.tile([C, G], f32, tag="sel")
                        nc.vector.tensor_tensor(
                            out=sel[:],
                            in0=bcol[:, w:w + 1].to_broadcast([C, G]),
                            in1=iota[:, 0:G],
                            op=mybir.AluOpType.is_equal)
                        nc.tensor.matmul(out=ps_pool_acc[:], lhsT=x2t[:],
                                         rhs=sel[:],
                                         start=(w == 0), stop=(w == W - 1))

            rg = [list(range(NCORES))]
            # ===== layer 1 =====
            dense_phase(xT, w1, 1, h1res, ag1l_in, ag1h_in)
            nc.gpsimd.collective_compute(
                "AllGather", mybir.AluOpType.bypass, replica_groups=rg,
                ins=[ag1l_in.opt()], outs=[ag1l_out.opt()])
            nc.gpsimd.collective_compute(
                "AllGather", mybir.AluOpType.bypass, replica_groups=rg,
                ins=[ag1h_in.opt()], outs=[ag1h_out.opt()])
            edge_layer(ag1l_out, ag1h_out, h1res, 1, bias1, x1res, None)

            # ===== layer 2 =====
            dense_phase(x1res, w2, 2, h2res, ag2l_in, ag2h_in)
            nc.gpsimd.collective_compute(
                "AllGather", mybir.AluOpType.bypass, replica_groups=rg,
                ins=[ag2l_in.opt()], outs=[ag2l_out.opt()])
            nc.gpsimd.collective_compute(
                "AllGather", mybir.AluOpType.bypass, replica_groups=rg,
                ins=[ag2h_in.opt()], outs=[ag2h_out.opt()])
            ps_pool = pst.tile([C, G], f32, space="PSUM")
            edge_layer(ag2l_out, ag2h_out, h2res, 2, bias2, None, ps_pool)

            # ===== pooling all-reduce + final linear =====
            poolT = res.tile([C, G], f32)
            nc.vector.tensor_copy(out=poolT[:], in_=ps_pool[:])
            nc.gpsimd.dma_start(out=ar_in[:], in_=poolT[:])
            nc.gpsimd.collective_compute(
                "AllReduce", mybir.AluOpType.add, replica_groups=rg,
                ins=[ar_in.opt()], outs=[ar_out.opt()])
            poolS = res.tile([C, G], f32)
            nc.sync.dma_start(out=poolS[:], in_=ar_out[:])
            ps_f = psd.tile([G, C_OUT], f32, space="PSUM", tag="pd")
            nc.tensor.matmul(out=ps_f[:], lhsT=poolS[:], rhs=wlin[:],
                             start=True, stop=True)
            fin = res.tile([G, C_OUT], f32)
            nc.vector.tensor_scalar_mul(fin[:], in0=ps_f[:], scalar1=icnt[:])
            nc.vector.tensor_add(out=fin[:], in0=fin[:], in1=blinb[:])
            nc.sync.dma_start(out=out_t[:], in_=fin[:])

    nc.compile()
    nc.m = get_hw_module(nc.m)
    return nc


def _preprocess(edge_index, batch):
    """Host-side integer graph preprocessing: degrees, edge-cut sharding,
    dst-window bucketing, int16 gather index layout."""
    src = np.asarray(edge_index[0], dtype=np.int64)
    dst = np.asarray(edge_index[1], dtype=np.int64)
    batch = np.asarray(batch, dtype=np.int64)

    deg = np.bincount(dst, minlength=N).astype(np.float64) + 1.0
    dinv = (1.0 / np.sqrt(deg)).astype(np.float32)
    counts = np.bincount(batch, minlength=G).astype(np.float64)
    inv_cnt = (1.0 / np.maximum(counts, 1.0)).astype(np.float32)

    # order edges by dst (stable) once; then slice per core
    order = np.argsort(dst, kind="stable")
    src_s = src[order]
    dst_s = dst[order]
    core_lo = np.searchsorted(dst_s, np.arange(NCORES) * NLOC)
    core_hi = np.searchsorted(dst_s, (np.arange(NCORES) + 1) * NLOC)

    # per (core, window): lists split by src half
    per_core = []
    a_cnt = np.zeros((NCORES, W), np.int64)
    b_cnt = np.zeros((NCORES, W), np.int64)
    for c in range(NCORES):
        s = src_s[core_lo[c]:core_hi[c]]
        d = dst_s[core_lo[c]:core_hi[c]] - c * NLOC
        win = d >> 7
        wlo = np.searchsorted(win, np.arange(W))
        whi = np.searchsorted(win, np.arange(W) + 1)
        owner = s // NLOC
        pos = s - owner * NLOC
        is_lo_all = pos < HALF
        row_all = np.where(is_lo_all, owner * HALF + pos,
                           owner * HALF + (pos - HALF))
        wins = []
        for w in range(W):
            sl = slice(wlo[w], whi[w])
            rw = row_all[sl]
            dw = d[sl] - w * 128
            il = is_lo_all[sl]
            wins.append((rw[il], dw[il], rw[~il], dw[~il]))
            a_cnt[c, w] = int(il.sum())
            b_cnt[c, w] = len(rw) - a_cnt[c, w]
        per_core.append(wins)

    a_chunks = [int(-(-a_cnt[:, w].max() // 128)) for w in range(W)]
    b_chunks = [int(-(-b_cnt[:, w].max() // 128)) for w in range(W)]

    # gather call plan (uniform across cores): (half, chunk_count, idx_col, queue)
    call_plan = []
    total_chunks = 0
    idx_col = 0
    qn = 0
    for w in range(W):
        calls = []
        for half, nch in ((0, a_chunks[w]), (1, b_chunks[w])):
            left = nch
            while left > 0:
                take = min(left, MAX_CALL_CHUNKS)
                calls.append((half, take, idx_col, qn % NQ))
                qn += 1
                idx_col += take * 8
                left -= take
        call_plan.append(calls)
        total_chunks += a_chunks[w] + b_chunks[w]
    total_idxcols = idx_col

    # per-core idx16 / drel tensors in program layout
    idx_arrs = []
    drel_arrs = []
    for c in range(NCORES):
        idx_t = np.zeros((C, total_idxcols), np.int16)
        drel_t = np.full((C, total_chunks), -1.0, np.float32)
        cb = 0
        for w in range(W):
            sw_lo, dw_lo, sw_hi, dw_hi = per_core[c][w]
            for half, nch, (sw, dw) in ((0, a_chunks[w], (sw_lo, dw_lo)),
                                        (1, b_chunks[w], (sw_hi, dw_hi))):
                if nch == 0:
                    continue
                slots = nch * 128
                flat_i = np.zeros(slots, np.int16)
                flat_i[:len(sw)] = sw.astype(np.int16)
                flat_d = np.full(slots, -1.0, np.float32)
                flat_d[:len(dw)] = dw.astype(np.float32)
                # drel columns: chunk j column cb+j, partition p = slot j*128+p
                drel_t[:, cb:cb + nch] = flat_d.reshape(nch, 128).T
                cb += nch
            # idx cols follow the call plan for this window
        # fill idx16 per call (needs flat lists again, per call slices)
        cb2 = 0
        for w in range(W):
            sw_lo, dw_lo, sw_hi, dw_hi = per_core[c][w]
            flats = {}
            for half, nch, sw in ((0, a_chunks[w], sw_lo), (1, b_chunks[w], sw_hi)):
                slots = nch * 128
                fi = np.zeros(slots, np.int16)
                fi[:len(sw)] = sw.astype(np.int16)
                flats[half] = fi
            consumed = {0: 0, 1: 0}
            for (half, take, col0, _q) in call_plan[w]:
                seg = flats[half][consumed[half] * 128:(consumed[half] + take) * 128]
                consumed[half] += take
                wrap = seg.reshape(take * 8, 16).T           # [16, take*8]
                idx_t[:, col0:col0 + take * 8] = np.tile(wrap, (8, 1))
        idx_arrs.append(idx_t)
        drel_arrs.append(drel_t)

    return (dinv, inv_cnt, batch, a_chunks, b_chunks, call_plan,
            total_chunks, total_idxcols, idx_arrs, drel_arrs)


def kernel(**inputs):
    from concourse import bass_utils

    x = np.asarray(inputs["x"], dtype=np.float32)
    (dinv, inv_cnt, batch, a_chunks, b_chunks, call_plan, total_chunks,
     total_idxcols, idx_arrs, drel_arrs) = _preprocess(
        np.asarray(inputs["edge_index"]), np.asarray(inputs["batch"]))

    key = (tuple(a_chunks), tuple(b_chunks),
           tuple(tuple(c) for cp in call_plan for c in cp))
    if key not in _CACHE:
        _CACHE.clear()
        _CACHE[key] = _build_program(a_chunks, b_chunks, call_plan,
                                     total_chunks, total_idxcols)
    nc = _CACHE[key]

    iota = np.tile(np.arange(C, dtype=np.float32), (C, 1))
    b1t = np.tile(np.asarray(inputs["b1e"], np.float32), (C, 1))
    b2t = np.tile(np.asarray(inputs["b2e"], np.float32), (C, 1))
    blinb = np.tile(np.asarray(inputs["blin"], np.float32), (G, 1))

    in_maps = []
    for c in range(NCORES):
        lo = c * NLOC
        xT = np.zeros((C, NPAD), np.float32)
        xT[:, :NLOC] = x[lo:lo + NLOC].T
        dv = np.zeros((C, W), np.float32)
        dv_flat = np.zeros(NPAD, np.float32)
        dv_flat[:NLOC] = dinv[lo:lo + NLOC]
        dv[:] = dv_flat.reshape(W, 128).T
        bc = np.full((C, W), -1.0, np.float32)
        bc_flat = np.full(NPAD, -1.0, np.float32)
        bc_flat[:NLOC] = batch[lo:lo + NLOC].astype(np.float32)
        bc[:] = bc_flat.reshape(W, 128).T
        in_maps.append({
            "xT": xT, "idx16": idx_arrs[c], "drel": drel_arrs[c],
            "iota": iota, "dinvw": dv, "batchcol": bc,
            "bias1t": b1t, "bias2t": b2t,
            "w1e": np.asarray(inputs["W1e"], np.float32),
            "w2e": np.asarray(inputs["W2e"], np.float32),
            "wlin": np.asarray(inputs["Wlin"], np.float32),
            "blinb": blinb, "invcnt": inv_cnt.reshape(G, 1),
        })

    trace = bool(inputs.get("_trace", False))
    res = bass_utils.run_bass_kernel_spmd(nc, in_maps,
                                          core_ids=list(range(NCORES)),
                                          trace=trace)
    kernel._last = res
    return np.asarray(res.results[0]["out"], dtype=np.float32)


# revision 7
# speedup vs baseline: 1.2580x; 1.2580x over previous
"""Trainium2 Bass kernel for the DiffPool-style GCN forward pass.

Computation (dead softmax/pool branches of the reference are skipped — their
outputs are unused):
    x1 = relu(Dhalf (A+I) Dhalf (x @ W1e) + b1e)
    x2 = relu(Dhalf (A+I) Dhalf (x1 @ W2e) + b2e)
    out = (graph_mean_pool(x2) @ Wlin) + blin          -> [64, 10] fp32

Normalization folds into node-level row scalings: with h' = dinv * (x @ W),
agg = dinv * (scatter_sum(h'[src] -> dst) + h') + b
    = dinv * scatter_sum + hb,   hb := dinv*h' + b   (precomputed per node).

Distribution: nodes (and incident edges, bucketed by dst) are sharded over
8 NeuronCores; 128x128 weights replicated; the full h' table is built with one
AllGather per layer (Shared output); per-graph mean-pool partials are combined
with an AllReduce.

Per-core edge pipeline: edges are sorted by dst into 128-node aligned windows;
h'[src] rows stream in via batched dma_gather (1024 rows/call, 4 SWDGE queues,
calls packed across windows); a one-hot [edge x window-node] matrix built on
DVE via broadcast is_equal turns the scatter-add into PE matmuls accumulating
in PSUM. int16 gather indices require <32768-row tables, so the table is used
as an offset-0 lo slice plus a DRAM copy of the hi half.
"""

import numpy as np

N = 50000
E = 800000
G = 64
C = 128
C_OUT = 10
NCORES = 8
NLOC = N // NCORES          # 6250
W = (NLOC + 127) // 128     # 49 windows of 128 dst nodes
NPAD = W * 128              # 6272
HALF = 25000                # lo table = global nodes [0, 25000)
MAX_CALL_CHUNKS = 8         # 1024 rows per dma_gather call
NQ = 4                      # SWDGE queues

_CACHE = {}


def _build_program(plan):
    import concourse.bass as bass
    import concourse.bacc as bacc
    import concourse.mybir as mybir
    import concourse.tile as tile
    from concourse import library_config
    from concourse.bass_interp import get_hw_module
    from concourse.tile_rust import add_dep_helper
    from concourse.masks import make_identity

    f32 = mybir.dt.float32
    i16 = mybir.dt.int16
    Relu = mybir.ActivationFunctionType.Relu
    Copy = mybir.ActivationFunctionType.Copy

    a_chunks = plan["a_chunks"]
    b_chunks = plan["b_chunks"]
    calls = plan["calls"]            # list of (half, start_chunk, n_chunks, idx_col)
    win_lo_base = plan["win_lo_base"]
    win_hi_base = plan["win_hi_base"]
    TL = plan["TL"]                  # total lo chunks
    total_chunks = plan["total_chunks"]
    total_idxcols = plan["total_idxcols"]

    nc = bacc.Bacc("TRN2", target_bir_lowering=False, debug=False,
                   num_devices=NCORES, num_swdge_queues=NQ)

    # ---- I/O ----
    xT_in = nc.dram_tensor("xT", [C, NPAD], f32, kind="ExternalInput")
    idx_in = nc.dram_tensor("idx16", [C, total_idxcols], i16, kind="ExternalInput")
    drel_in = nc.dram_tensor("drel", [C, total_chunks], f32, kind="ExternalInput")
    iota_in = nc.dram_tensor("iota", [C, C], f32, kind="ExternalInput")
    dinv_in = nc.dram_tensor("dinvw", [C, W], f32, kind="ExternalInput")
    bcol_in = nc.dram_tensor("batchcol", [C, W], f32, kind="ExternalInput")
    b1_in = nc.dram_tensor("bias1t", [C, C], f32, kind="ExternalInput")
    b2_in = nc.dram_tensor("bias2t", [C, C], f32, kind="ExternalInput")
    w1_in = nc.dram_tensor("w1e", [C, C], f32, kind="ExternalInput")
    w2_in = nc.dram_tensor("w2e", [C, C], f32, kind="ExternalInput")
    wlin_in = nc.dram_tensor("wlin", [C, C_OUT], f32, kind="ExternalInput")
    blin_in = nc.dram_tensor("blinb", [G, C_OUT], f32, kind="ExternalInput")
    icnt_in = nc.dram_tensor("invcnt", [G, 1], f32, kind="ExternalInput")
    out_t = nc.dram_tensor("out", [G, C_OUT], f32, kind="ExternalOutput")

    with tile.TileContext(nc) as tc:
        with tc.tile_pool(name="res", bufs=1) as res, \
             tc.tile_pool(name="gp", bufs=8) as gp, \
             tc.tile_pool(name="ohp", bufs=8) as ohp, \
             tc.tile_pool(name="tmp", bufs=6) as tmpp, \
             tc.tile_pool(name="hx", bufs=4) as hxp, \
             tc.tile_pool(name="selp", bufs=4) as selp, \
             tc.tile_pool(name="psw", bufs=3, space="PSUM") as psw, \
             tc.tile_pool(name="psd", bufs=2, space="PSUM") as psd, \
             tc.tile_pool(name="pst", bufs=1, space="PSUM") as pst, \
             tc.tile_pool(name="dram", bufs=1, space="DRAM") as dram:

            lib = nc.gpsimd.load_library(library_config.mlp)

            # ---- residents ----
            def load_res(name, src, shape, dt=f32):
                t = res.tile(shape, dt, tag=name)
                nc.sync.dma_start(out=t[:], in_=src[:])
                return t

            xT = load_res("r_xT", xT_in, [C, NPAD])
            idx16 = load_res("r_idx", idx_in, [C, total_idxcols], i16)
            drel = load_res("r_drel", drel_in, [C, total_chunks])
            iota = load_res("r_iota", iota_in, [C, C])
            dinvw = load_res("r_dinv", dinv_in, [C, W])
            bcol = load_res("r_bcol", bcol_in, [C, W])
            bias1 = load_res("r_b1", b1_in, [C, C])
            bias2 = load_res("r_b2", b2_in, [C, C])
            w1 = load_res("r_w1", w1_in, [C, C])
            w2 = load_res("r_w2", w2_in, [C, C])
            wlin = load_res("r_wlin", wlin_in, [C, C_OUT])
            blinb = load_res("r_blin", blin_in, [G, C_OUT])
            icnt = load_res("r_icnt", icnt_in, [G, 1])
            ident = res.tile([C, C], f32)
            make_identity(nc, ident[:])

            hb1 = res.tile([C, NPAD], f32)   # dinv^2*(x@W1) + b1, node-major blocks
            hb2 = res.tile([C, NPAD], f32)

            # ---- collective / table buffers ----
            ag1_in = dram.tile([NLOC, C], f32)
            ag2_in = dram.tile([NLOC, C], f32)
            ag1_out = dram.tile([N, C], f32)
            ag2_out = dram.tile([N, C], f32)
            hi1 = dram.tile([N - HALF, C], f32)
            hi2 = dram.tile([N - HALF, C], f32)
            ar_in = dram.tile([C, G], f32)
            ar_out = dram.tile([C, G], f32)
            rg = [list(range(NCORES))]

            def dense_block(b, lhsT, wt, ag_in, hb, bias):
                """ps = lhsT.T @ wt; h' = dinv*ps -> ship rows; hb = dinv*h'+bias."""
                cols = slice(b * 128, (b + 1) * 128)
                ps = psd.tile([C, C], f32, space="PSUM", tag="pd")
                nc.tensor.matmul(out=ps[:], lhsT=lhsT, rhs=wt[:],
                                 start=True, stop=True)
                ht = hxp.tile([C, C], f32, tag="ht")
                nc.scalar.activation(ht[:], ps[:], Copy,
                                     scale=dinvw[:, b:b + 1])
                r0 = b * 128
                r1 = min(r0 + 128, NLOC)
                nc.sync.dma_start(out=ag_in[r0:r1, :], in_=ht[0:r1 - r0, :])
                t = tmpp.tile([C, C], f32, tag="hbT")
                nc.scalar.activation(t[:], ht[:], Copy,
                                     scale=dinvw[:, b:b + 1])
                nc.vector.tensor_add(out=hb[:, cols], in0=t[:], in1=bias[:])

            def dense2_block(b, x1t, w2_, ident_):
                pt = psd.tile([C, C], f32, space="PSUM", tag="tps")
                nc.tensor.transpose(out=pt[:], in_=x1t[:], identity=ident_[:])
                xts = tmpp.tile([C, C], f32, tag="xts")
                nc.vector.tensor_copy(out=xts[:], in_=pt[:])
                dense_block(b, xts[:], w2_, ag2_in, hb2, bias2)

            def edge_layer(lo_ap, hi_ap, hb, layer, ps_pool_acc):
                tiles = {}          # call id -> (gt, oh, half, start, nch)
                next_call = [0]

                def ensure_chunk(half, s):
                    while True:
                        for ci, (gt, oh, h2, st, nch) in tiles.items():
                            if h2 == half and st <= s < st + nch:
                                return gt, oh, s - st
                        ci = next_call[0]
                        assert ci < len(calls), (half, s)
                        h2, st, nch, col = calls[ci]
                        gt = gp.tile([C, MAX_CALL_CHUNKS * C], f32, tag="g")
                        src_ap = lo_ap if h2 == 0 else hi_ap
                        ni = nch * 128
                        gi = nc.gpsimd.dma_gather(
                            gt[:, 0:nch * C].rearrange("p (k d) -> p k d", d=C),
                            src_ap, idx16[:, col:col + nch * 8],
                            ni, ni, C, single_packet=True, queue_num=ci % NQ)
                        add_dep_helper(gi.ins, lib.ins, False, "needs mlp lib")
                        oh = ohp.tile([C, MAX_CALL_CHUNKS * C], f32, tag="oh")
                        gstart = st if h2 == 0 else TL + st
                        dcols = drel[:, gstart:gstart + nch]
                        nc.vector.tensor_tensor(
                            out=oh[:, 0:nch * C].rearrange("p (k m) -> p k m", m=C),
                            in0=dcols.unsqueeze(2).to_broadcast([C, nch, C]),
                            in1=iota[:].unsqueeze(1).to_broadcast([C, nch, C]),
                            op=mybir.AluOpType.is_equal)
                        tiles[ci] = (gt, oh, h2, st, nch)
                        next_call[0] += 1

                for w in range(W):
                    aw, bw = a_chunks[w], b_chunks[w]
                    cw = aw + bw
                    ps = psw.tile([C, C], f32, space="PSUM", tag="pw")
                    j = 0
                    for half, base, cnt in ((0, win_lo_base[w], aw),
                                            (1, win_hi_base[w], bw)):
                        for k in range(cnt):
                            gt, oh, off = ensure_chunk(half, base + k)
                            nc.tensor.matmul(
                                out=ps[:],
                                lhsT=oh[:, off * C:(off + 1) * C],
                                rhs=gt[:, off * C:(off + 1) * C],
                                start=(j == 0), stop=(j == cw - 1))
                            j += 1
                    # epilogue: relu(dinv*S + hb)
                    cols = slice(w * 128, (w + 1) * 128)
                    t = tmpp.tile([C, C], f32, tag="ep")
                    nc.scalar.activation(t[:], ps[:], Copy,
                                         scale=dinvw[:, w:w + 1])
                    nc.vector.tensor_add(out=t[:], in0=t[:], in1=hb[:, cols])
                    xt = hxp.tile([C, C], f32, tag="xt")
                    nc.scalar.activation(xt[:], t[:], Relu)
                    if layer == 1:
                        dense2_block(w, xt, w2, ident)
                    else:
                        sel = selp.tile([C, G], f32, tag="sel")
                        nc.vector.tensor_tensor(
                            out=sel[:],
                            in0=bcol[:, w:w + 1].to_broadcast([C, G]),
                            in1=iota[:, 0:G],
                            op=mybir.AluOpType.is_equal)
                        nc.tensor.matmul(out=ps_pool_acc[:], lhsT=xt[:],
                                         rhs=sel[:],
                                         start=(w == 0), stop=(w == W - 1))

            # ===== layer 1 dense =====
            for b in range(W):
                dense_block(b, xT[:, b * 128:(b + 1) * 128], w1, ag1_in,
                            hb1, bias1)
            nc.gpsimd.collective_compute(
                "AllGather", mybir.AluOpType.bypass, replica_groups=rg,
                ins=[ag1_in.opt()], outs=[ag1_out.opt()])
            nc.sync.dma_start(out=hi1[:], in_=ag1_out[HALF:N, :])

            # ===== layer 1 edges (+ interleaved layer-2 dense) =====
            edge_layer(ag1_out[0:HALF, :], hi1[:], hb1, 1, None)

            nc.gpsimd.collective_compute(
                "AllGather", mybir.AluOpType.bypass, replica_groups=rg,
                ins=[ag2_in.opt()], outs=[ag2_out.opt()])
            nc.sync.dma_start(out=hi2[:], in_=ag2_out[HALF:N, :])

            # ===== layer 2 edges + pooling =====
            ps_pool = pst.tile([C, G], f32, space="PSUM")
            edge_layer(ag2_out[0:HALF, :], hi2[:], hb2, 2, ps_pool)

            # ===== pooled all-reduce + final linear =====
            poolT = res.tile([C, G], f32)
            nc.vector.tensor_copy(out=poolT[:], in_=ps_pool[:])
            nc.gpsimd.dma_start(out=ar_in[:], in_=poolT[:])
            nc.gpsimd.collective_compute(
                "AllReduce", mybir.AluOpType.add, replica_groups=rg,
                ins=[ar_in.opt()], outs=[ar_out.opt()])
            poolS = res.tile([C, G], f32)
            nc.sync.dma_start(out=poolS[:], in_=ar_out[:])
            ps_f = psd.tile([G, C_OUT], f32, space="PSUM", tag="pd")
            nc.tensor.matmul(out=ps_f[:], lhsT=poolS[:], rhs=wlin[:],
                             start=True, stop=True)
            fin = res.tile([G, C_OUT], f32)
            nc.vector.tensor_scalar_mul(fin[:], in0=ps_f[:], scalar1=icnt[:])
            nc.vector.tensor_add(out=fin[:], in0=fin[:], in1=blinb[:])
            nc.sync.dma_start(out=out_t[:], in_=fin[:])

    nc.compile()
    nc.m = get_hw_module(nc.m)
    return nc


def _preprocess(edge_index, batch):
    """Host-side integer graph preprocessing: degrees, edge-cut sharding,
    dst-window bucketing, packed int16 gather-call layout."""
    src = np.asarray(edge_index[0], dtype=np.int64)
    dst = np.asarray(edge_index[1], dtype=np.int64)
    batch = np.asarray(batch, dtype=np.int64)

    deg = np.bincount(dst, minlength=N).astype(np.float64) + 1.0
    dinv = (1.0 / np.sqrt(deg)).astype(np.float32)
    counts = np.bincount(batch, minlength=G).astype(np.float64)
    inv_cnt = (1.0 / np.maximum(counts, 1.0)).astype(np.float32)

    order = np.argsort(dst, kind="stable")
    src_s = src[order]
    dst_s = dst[order]
    core_lo = np.searchsorted(dst_s, np.arange(NCORES) * NLOC)
    core_hi = np.searchsorted(dst_s, (np.arange(NCORES) + 1) * NLOC)

    per_core = []
    a_cnt = np.zeros((NCORES, W), np.int64)
    b_cnt = np.zeros((NCORES, W), np.int64)
    for c in range(NCORES):
        s = src_s[core_lo[c]:core_hi[c]]
        d = dst_s[core_lo[c]:core_hi[c]] - c * NLOC
        win = d >> 7
        wlo = np.searchsorted(win, np.arange(W))
        whi = np.searchsorted(win, np.arange(W) + 1)
        wins = []
        for w in range(W):
            sl = slice(wlo[w], whi[w])
            sw = s[sl]
            dw = d[sl] - w * 128
            il = sw < HALF
            wins.append((sw[il], dw[il], sw[~il] - HALF, dw[~il]))
            a_cnt[c, w] = int(il.sum())
            b_cnt[c, w] = len(sw) - a_cnt[c, w]
        per_core.append(wins)

    a_chunks = [int(-(-a_cnt[:, w].max() // 128)) for w in range(W)]
    b_chunks = [int(-(-b_cnt[:, w].max() // 128)) for w in range(W)]
    win_lo_base = np.concatenate([[0], np.cumsum(a_chunks)])[:W].astype(int).tolist()
    win_hi_base = np.concatenate([[0], np.cumsum(b_chunks)])[:W].astype(int).tolist()
    TL = int(sum(a_chunks))
    TH = int(sum(b_chunks))
    total_chunks = TL + TH

    # packed gather calls: groups of <=8 consecutive chunks within each
    # half-stream, listed in per-window consumption order so the kernel can
    # emit them lazily right before their first consumer.
    calls = []
    idx_col = 0
    lo_done = hi_done = 0
    for w in range(W):
        need_lo = win_lo_base[w] + a_chunks[w]
        while lo_done < need_lo:
            take = min(MAX_CALL_CHUNKS, TL - lo_done)
            calls.append((0, lo_done, take, idx_col))
            idx_col += take * 8
            lo_done += take
        need_hi = win_hi_base[w] + b_chunks[w]
        while hi_done < need_hi:
            take = min(MAX_CALL_CHUNKS, TH - hi_done)
            calls.append((1, hi_done, take, idx_col))
            idx_col += take * 8
            hi_done += take
    total_idxcols = idx_col

    plan = {"a_chunks": a_chunks, "b_chunks": b_chunks, "calls": calls,
            "win_lo_base": win_lo_base, "win_hi_base": win_hi_base,
            "TL": TL, "total_chunks": total_chunks,
            "total_idxcols": total_idxcols}

    # per-core flat chunk arrays (idx + drel) in stream order
    idx_arrs = []
    drel_arrs = []
    for c in range(NCORES):
        lo_idx = np.zeros(TL * 128, np.int16)
        hi_idx = np.zeros(TH * 128, np.int16)
        drel_t = np.full((128, total_chunks), -1.0, np.float32)
        for w in range(W):
            sw_lo, dw_lo, sw_hi, dw_hi = per_core[c][w]
            o = win_lo_base[w] * 128
            lo_idx[o:o + len(sw_lo)] = sw_lo.astype(np.int16)
            fl = np.full(a_chunks[w] * 128, -1.0, np.float32)
            fl[:len(dw_lo)] = dw_lo.astype(np.float32)
            drel_t[:, win_lo_base[w]:win_lo_base[w] + a_chunks[w]] = \
                fl.reshape(a_chunks[w], 128).T
            o = win_hi_base[w] * 128
            hi_idx[o:o + len(sw_hi)] = sw_hi.astype(np.int16)
            fh = np.full(b_chunks[w] * 128, -1.0, np.float32)
            fh[:len(dw_hi)] = dw_hi.astype(np.float32)
            drel_t[:, TL + win_hi_base[w]:TL + win_hi_base[w] + b_chunks[w]] = \
                fh.reshape(b_chunks[w], 128).T
        idx_t = np.zeros((128, total_idxcols), np.int16)
        for half, s0, take, col in calls:
            seg = (lo_idx if half == 0 else hi_idx)[s0 * 128:(s0 + take) * 128]
            wrap = seg.reshape(take * 8, 16).T
            idx_t[:, col:col + take * 8] = np.tile(wrap, (8, 1))
        idx_arrs.append(idx_t)
        drel_arrs.append(drel_t)

    return dinv, inv_cnt, batch, plan, idx_arrs, drel_arrs


def kernel(**inputs):
    from concourse import bass_utils

    x = np.asarray(inputs["x"], dtype=np.float32)
    dinv, inv_cnt, batch, plan, idx_arrs, drel_arrs = _preprocess(
        np.asarray(inputs["edge_index"]), np.asarray(inputs["batch"]))

    key = (tuple(plan["a_chunks"]), tuple(plan["b_chunks"]))
    if key not in _CACHE:
        _CACHE.clear()
        _CACHE[key] = _build_program(plan)
    nc = _CACHE[key]

    iota = np.tile(np.arange(C, dtype=np.float32), (C, 1))
    b1t = np.tile(np.asarray(inputs["b1e"], np.float32), (C, 1))
    b2t = np.tile(np.asarray(inputs["b2e"], np.float32), (C, 1))
    blinb = np.tile(np.asarray(inputs["blin"], np.float32), (G, 1))

    in_maps = []
    for c in range(NCORES):
        lo = c * NLOC
        xT = np.zeros((C, NPAD), np.float32)
        xT[:, :NLOC] = x[lo:lo + NLOC].T
        dv_flat = np.zeros(NPAD, np.float32)
        dv_flat[:NLOC] = dinv[lo:lo + NLOC]
        bc_flat = np.full(NPAD, -1.0, np.float32)
        bc_flat[:NLOC] = batch[lo:lo + NLOC].astype(np.float32)
        in_maps.append({
            "xT": xT, "idx16": idx_arrs[c], "drel": drel_arrs[c],
            "iota": iota,
            "dinvw": dv_flat.reshape(W, 128).T.copy(),
            "batchcol": bc_flat.reshape(W, 128).T.copy(),
            "bias1t": b1t, "bias2t": b2t,
            "w1e": np.asarray(inputs["W1e"], np.float32),
            "w2e": np.asarray(inputs["W2e"], np.float32),
            "wlin": np.asarray(inputs["Wlin"], np.float32),
            "blinb": blinb, "invcnt": inv_cnt.reshape(G, 1),
        })

    trace = bool(inputs.get("_trace", False))
    res = bass_utils.run_bass_kernel_spmd(nc, in_maps,
                                          core_ids=list(range(NCORES)),
                                          trace=trace)
    kernel._last = res
    return np.asarray(res.results[0]["out"], dtype=np.float32)
